# revision 12
# baseline (speedup 1.0000x reference)
"""Point Transformer last-2-layers kernel for TRN2, 8 NeuronCores.

Data-parallel over batch: 2 items per core, weights replicated.
Per item-layer on a core (C=256, N=2048):
  q = Wq@x, k = Wk@x (fp32 PE)
  energy blocks e_nb = q[:,nb].T @ k   (16 blocks of [128,2048], fp32)
  rowmax via DVE (negated), E_raw = exp(e - mx) in bf16 (ACT, accum_out -> R)
  v^T = x^T Wv^T + bv (ones-row matmul), scaled by 1/R at evict -> uT fp16
  colsum s = sum_blocks recipR_f16.T @ E_raw;  r = 1/(1e-9+s)
  xr = uT.T @ E_raw (accumulated over blocks); h = x - xr * broadcast(r)
  h2 = Wt @ h (fp32); S1/S2 row-sums via ACT accum_out
BatchNorm stats: one AllReduce of [128,4] (S1,S2 per channel) per layer;
bt is skipped (provably cancels through training-mode BN).
y = x + relu(scale*h2 + shift); layer2 consumes y1 in-place.
"""

import os
import numpy as np

from concourse import bass, bacc, tile
from concourse.bass_utils import run_bass_kernel_spmd
import concourse.mybir as mybir

F32 = mybir.dt.float32
F32R = mybir.dt.float32r
BF16 = mybir.dt.bfloat16
F16 = mybir.dt.float16


def _r(ap):
    return ap.bitcast(F32R)
AX = mybir.AxisListType
OP = mybir.AluOpType
ACTF = mybir.ActivationFunctionType

N_CORES = 8
B = 16
PB = B // N_CORES  # items per core
C = 256
N = 2048
NB = N // 128      # 16 n-blocks
CNT = B * N        # BN count over (batch, points)
BN_EPS = 1e-5

TRACE = False
LAST_EXEC_NS = None

_NC = None


def _emit(tc, n_cores):
    nc = tc.nc
    x_d = nc.dram_tensor("x", [PB, C, N], F32, kind="ExternalInput")
    y_d = nc.dram_tensor("y", [PB, 2 * C, N], F32, kind="ExternalOutput")
    wq_d, wk_d, wv_d, wt_d, bv_d, gb_d = {}, {}, {}, {}, {}, {}
    for L in (1, 2):
        wq_d[L] = nc.dram_tensor(f"wq{L}", [2, 128, 128], F32, kind="ExternalInput")
        wk_d[L] = nc.dram_tensor(f"wk{L}", [2, 128, 128], F32, kind="ExternalInput")
        wv_d[L] = nc.dram_tensor(f"wv{L}", [2, 128, 256], F32, kind="ExternalInput")
        wt_d[L] = nc.dram_tensor(f"wt{L}", [2, 128, 256], F32, kind="ExternalInput")
        bv_d[L] = nc.dram_tensor(f"bv{L}", [1, 256], F32, kind="ExternalInput")
        gb_d[L] = nc.dram_tensor(f"gb{L}", [128, 4], F32, kind="ExternalInput")

    with tc.tile_pool(name="sb", bufs=1) as sb, \
         tc.tile_pool(name="ps", bufs=1, space="PSUM") as ps, \
         tc.tile_pool(name="dr", bufs=1, space="DRAM") as dr:
        x_sb = sb.tile([128, PB, 2, N], F32, name="x_sb")
        wq_sb, wk_sb, wv_sb, wt_sb, bv_sb, gb_sb = {}, {}, {}, {}, {}, {}
        for L in (1, 2):
            wq_sb[L] = sb.tile([128, 2, 128], F32, name=f"wq_sb{L}")
            wk_sb[L] = sb.tile([128, 2, 128], F32, name=f"wk_sb{L}")
            wv_sb[L] = sb.tile([128, 2, 256], F32, name=f"wv_sb{L}")
            wt_sb[L] = sb.tile([128, 2, 256], F32, name=f"wt_sb{L}")
            bv_sb[L] = sb.tile([1, 256], F32, name=f"bv_sb{L}")
            gb_sb[L] = sb.tile([128, 4], F32, name=f"gb_sb{L}")
        ones = sb.tile([1, 128], F32, name="ones")
        q_sb = sb.tile([128, N], F32, name="q_sb")   # reused as h chunk 0
        k_sb = sb.tile([128, N], F32, name="k_sb")   # reused as h chunk 1
        E_raw = sb.tile([128, NB, N], BF16, name="E_raw")
        uT = sb.tile([128, NB, 256], F16, name="uT")
        negmx = sb.tile([128, NB], F32, name="negmx")
        Rcol = sb.tile([128, NB], F32, name="Rcol")
        recipR = sb.tile([128, NB], F32, name="recipR")
        recipR_lp = sb.tile([128, NB], F16, name="recipR_lp")
        r_sb = sb.tile([1, N], F32, name="r_sb")
        rb_sb = sb.tile([128, 1024], F32, name="rb_sb")
        h2_sb = sb.tile([128, PB, 2, N], F32, name="h2_sb")
        stage = sb.tile([128, N], F32, name="stage")
        s1 = sb.tile([128, PB, 2], F32, name="s1")
        s2 = sb.tile([128, PB, 2], F32, name="s2")
        arbuf = sb.tile([128, 4], F32, name="arbuf")
        arres = sb.tile([128, 4], F32, name="arres")
        bnt = sb.tile([128, 10], F32, name="bnt")
        psA = ps.tile([128, N], F32, name="psA")
        psB = ps.tile([128, N], F32, name="psB")
        ccin, ccout = {}, {}
        for L in (1, 2):
            ccin[L] = dr.tile([128, 4], F32, name=f"ccin{L}")
            ccout[L] = dr.tile([128, 4], F32, name=f"ccout{L}")

        # ---- loads ----
        for item in range(PB):
            for c_ in (0, 1):
                nc.sync.dma_start(x_sb[:, item, c_, :],
                                  x_d[item, c_ * 128:(c_ + 1) * 128, :])
        for L in (1, 2):
            for c_ in (0, 1):
                nc.sync.dma_start(wq_sb[L][:, c_, :], wq_d[L][c_])
                nc.sync.dma_start(wk_sb[L][:, c_, :], wk_d[L][c_])
                nc.sync.dma_start(wv_sb[L][:, c_, :], wv_d[L][c_])
                nc.sync.dma_start(wt_sb[L][:, c_, :], wt_d[L][c_])
            nc.sync.dma_start(bv_sb[L][:], bv_d[L][:])
            nc.sync.dma_start(gb_sb[L][:], gb_d[L][:])
        nc.vector.memset(ones[:], 1.0)

        for L in (1, 2):
            for item in range(PB):
                # ---- QK ----
                for m in range(4):
                    sl = slice(m * 512, (m + 1) * 512)
                    for c_ in (0, 1):
                        nc.tensor.matmul(psA[:, sl], wq_sb[L][:, c_, :],
                                         x_sb[:, item, c_, sl],
                                         start=(c_ == 0), stop=(c_ == 1))
                        nc.tensor.matmul(psB[:, sl], wk_sb[L][:, c_, :],
                                         x_sb[:, item, c_, sl],
                                         start=(c_ == 0), stop=(c_ == 1))
                nc.scalar.activation(q_sb[:], psA[:], ACTF.Copy)
                nc.scalar.activation(k_sb[:], psB[:], ACTF.Copy)

                # ---- E blocks: energy, rowmax, exp, R ----
                for nb in range(NB):
                    pe = psA if nb % 2 == 0 else psB
                    for m in range(4):
                        sl = slice(m * 512, (m + 1) * 512)
                        nc.tensor.matmul(pe[:, sl],
                                         q_sb[:, nb * 128:(nb + 1) * 128],
                                         k_sb[:, sl], start=True, stop=True)
                    nc.vector.tensor_reduce(negmx[:, nb:nb + 1], pe[:],
                                            axis=AX.X, op=OP.max, negate=True)
                    nc.scalar.activation(E_raw[:, nb, :], pe[:], ACTF.Exp,
                                         bias=negmx[:, nb:nb + 1], scale=1.0,
                                         accum_out=Rcol[:, nb:nb + 1])
                nc.vector.reciprocal(recipR[:], Rcol[:])
                nc.vector.tensor_scalar_mul(recipR_lp[:], recipR[:], 1.0)

                # ---- V: vT = x^T Wv^T + bv, scaled by recipR at evict ----
                for nb in range(NB):
                    pv = (psA if nb % 2 == 0 else psB)[:, 0:256]
                    nsl = slice(nb * 128, (nb + 1) * 128)
                    nc.tensor.matmul(pv, x_sb[:, item, 0, nsl],
                                     wv_sb[L][:, 0, :],
                                     start=True, stop=False)
                    nc.tensor.matmul(pv, x_sb[:, item, 1, nsl],
                                     wv_sb[L][:, 1, :],
                                     start=False, stop=False)
                    nc.tensor.matmul(pv, ones[0:1, :], bv_sb[L][0:1, :],
                                     start=False, stop=True)
                    nc.vector.tensor_scalar(uT[:, nb, :], pv,
                                            recipR[:, nb:nb + 1], None, OP.mult)

                # ---- colsum s and r = 1/(1e-9+s) ----
                for m in range(4):
                    sl = slice(m * 512, (m + 1) * 512)
                    for nb in range(NB):
                        nc.tensor.matmul(psA[0:1, sl], recipR_lp[:, nb:nb + 1],
                                         E_raw[:, nb, sl],
                                         start=(nb == 0), stop=(nb == NB - 1))
                nc.vector.tensor_scalar_add(psA[0:1, :], psA[0:1, :], 1e-9)
                nc.vector.reciprocal(r_sb[0:1, :], psA[0:1, :])

                # ---- XR waves: xr = uT.T @ E_raw ; h = x - xr*r ----
                for ms in ((0, 1), (2, 3)):
                    for mi, m in enumerate(ms):
                        sl = slice(m * 512, (m + 1) * 512)
                        prb = psB[:, mi * 512:(mi + 1) * 512]
                        nc.tensor.matmul(prb, ones[0:1, :],
                                         r_sb[0:1, sl],
                                         start=True, stop=True)
                        rbs = rb_sb[:, mi * 512:(mi + 1) * 512]
                        nc.scalar.activation(rbs, prb, ACTF.Copy)
                        for c_ in (0, 1):
                            tsl = slice((2 * mi + c_) * 512,
                                        (2 * mi + c_ + 1) * 512)
                            pxr = psA[:, tsl]
                            for nb in range(NB):
                                nc.tensor.matmul(
                                    pxr, uT[:, nb, c_ * 128:(c_ + 1) * 128],
                                    E_raw[:, nb, sl],
                                    start=(nb == 0), stop=(nb == NB - 1))
                            h = q_sb if c_ == 0 else k_sb
                            nc.vector.tensor_tensor(stage[:, tsl], pxr, rbs,
                                                    OP.mult)
                            nc.vector.tensor_tensor(h[:, sl],
                                                    x_sb[:, item, c_, sl],
                                                    stage[:, tsl], OP.subtract)

                # ---- WT: h2 = Wt @ h, S1/S2 ----
                for och in (0, 1):
                    pw = psA if och == 0 else psB
                    for m in range(4):
                        sl = slice(m * 512, (m + 1) * 512)
                        for c_ in (0, 1):
                            h = q_sb if c_ == 0 else k_sb
                            nc.tensor.matmul(
                                pw[:, sl],
                                wt_sb[L][:, c_, och * 128:(och + 1) * 128],
                                h[:, sl], start=(c_ == 0), stop=(c_ == 1))
                    nc.scalar.activation(h2_sb[:, item, och, :], pw[:],
                                         ACTF.Copy,
                                         accum_out=s1[:, item, och:och + 1])
                    nc.scalar.activation(stage[:], h2_sb[:, item, och, :],
                                         ACTF.Square,
                                         accum_out=s2[:, item, och:och + 1])

            # ---- BN stats AllReduce ----
            nc.vector.tensor_tensor(arbuf[:, 0:2], s1[:, 0, :], s1[:, 1, :],
                                    OP.add)
            nc.vector.tensor_tensor(arbuf[:, 2:4], s2[:, 0, :], s2[:, 1, :],
                                    OP.add)
            nc.gpsimd.dma_start(ccin[L][:], arbuf[:])
            nc.gpsimd.collective_compute(
                "AllReduce", OP.add,
                replica_groups=[list(range(n_cores))],
                ins=[ccin[L].opt()], outs=[ccout[L].opt()])
            nc.gpsimd.dma_start(arres[:], ccout[L][:])

            # ---- BN math on [128,2] ----
            mean, var = bnt[:, 0:2], bnt[:, 2:4]
            scale, shift, tmp = bnt[:, 4:6], bnt[:, 6:8], bnt[:, 8:10]
            nc.vector.tensor_scalar_mul(mean, arres[:, 0:2], 1.0 / CNT)
            nc.vector.tensor_scalar_mul(var, arres[:, 2:4], 1.0 / CNT)
            nc.vector.tensor_tensor(tmp, mean, mean, OP.mult)
            nc.vector.tensor_tensor(var, var, tmp, OP.subtract)
            nc.vector.tensor_scalar_add(var, var, BN_EPS)
            nc.scalar.activation(tmp, var, ACTF.Sqrt)
            nc.vector.reciprocal(var, tmp)  # istd
            nc.vector.tensor_tensor(scale, gb_sb[L][:, 0:2], var, OP.mult)
            nc.vector.tensor_tensor(tmp, scale, mean, OP.mult)
            nc.vector.tensor_tensor(shift, gb_sb[L][:, 2:4], tmp, OP.subtract)

            # ---- apply: y = x + relu(scale*h2 + shift); write out ----
            for item in range(PB):
                for och in (0, 1):
                    nc.scalar.activation(stage[:], h2_sb[:, item, och, :],
                                         ACTF.Relu,
                                         bias=shift[:, och:och + 1],
                                         scale=scale[:, och:och + 1])
                    nc.vector.tensor_tensor(x_sb[:, item, och, :],
                                            x_sb[:, item, och, :], stage[:],
                                            OP.add)
                    off = (L - 1) * 256 + och * 128
                    nc.sync.dma_start(y_d[item, off:off + 128, :],
                                      x_sb[:, item, och, :])


def build_nc(n_cores=N_CORES, finalize=True):
    nc = bacc.Bacc("TRN2", target_bir_lowering=False, debug=False,
                   num_devices=n_cores)
    with tile.TileContext(nc) as tc:
        _emit(tc, n_cores)
    if finalize:
        nc.finalize()
    return nc


def _pack_weights(inputs):
    w = {}
    for L in (1, 2):
        Wq = np.asarray(inputs[f"Wq{L}"], np.float32)
        Wk = np.asarray(inputs[f"Wk{L}"], np.float32)
        Wv = np.asarray(inputs[f"Wv{L}"], np.float32)
        Wt = np.asarray(inputs[f"Wt{L}"], np.float32)
        bv = np.asarray(inputs[f"bv{L}"], np.float32)
        g = np.asarray(inputs[f"gamma{L}"], np.float32)
        b = np.asarray(inputs[f"beta{L}"], np.float32)
        w[f"wq{L}"] = np.ascontiguousarray(Wq.T.reshape(2, 128, 128))
        w[f"wk{L}"] = np.ascontiguousarray(Wk.T.reshape(2, 128, 128))
        w[f"wv{L}"] = np.ascontiguousarray(Wv.T.reshape(2, 128, 256))
        w[f"wt{L}"] = np.ascontiguousarray(Wt.T.reshape(2, 128, 256))
        w[f"bv{L}"] = np.ascontiguousarray(bv.reshape(1, 256))
        w[f"gb{L}"] = np.ascontiguousarray(
            np.stack([g[:128], g[128:], b[:128], b[128:]], axis=1))
    return w


def kernel(**inputs):
    global _NC, LAST_EXEC_NS
    if _NC is None:
        _NC = build_nc(N_CORES)
    x = np.asarray(inputs["x"], np.float32)
    w = _pack_weights(inputs)
    in_maps = []
    for core in range(N_CORES):
        m = dict(w)
        m["x"] = np.ascontiguousarray(x[core * PB:(core + 1) * PB])
        in_maps.append(m)
    res = run_bass_kernel_spmd(_NC, in_maps, list(range(N_CORES)),
                               trace=TRACE)
    LAST_EXEC_NS = res.exec_time_ns
    out = np.empty((B, 2 * C, N), np.float32)
    for core in range(N_CORES):
        out[core * PB:(core + 1) * PB] = np.asarray(res.results[core]["y"])
    return out
